# revision 19
# baseline (speedup 1.0000x reference)
"""BMN (Boundary Matching Network) forward pass as a Bass/Tile kernel on 8 trn2 cores.

Sharding: core i handles batch b = i//4 and duration-block j = i%4 (25 rows of
the [DM=100, T=100] BM map, +2 halo rows each side for the two 3x3 convs).

Key reformulation: the reference computes
    bm   = p @ sample_mask                      # [B,H1,NS,DM,T]  (huge)
    fmap = relu(einsum('bcnds,ocn->bods', bm, w_3d))
We instead contract w_3d with p first:
    r[n](t, o) = sum_c p[c,t] * w_3d[o,c,n]     # 32 small matmuls
    fmap[o, (d,s)] = relu(sum_n r[n].T @ mask[n] + b_3d)  # PSUM-accumulated over n
which avoids materializing bm and shrinks the contraction from (C*N)=8192
to (N*T)=3200.

All matmuls run in bf16 with fp32 PSUM accumulation. sample_mask / weights are
pre-cast to bf16 on the host (round-to-nearest-even, identical to an on-device
cast) to halve DMA. The mask is host-packed as [T, ntile, NS, 500] so each
N-tile group loads with 16KB-contiguous per-partition lines (DMA efficiency).
Grouped conv1ds are emitted as dense matmuls with block-diagonal weights.
"""

import numpy as np
import ml_dtypes

import concourse.bass as bass
import concourse.mybir as mybir
import concourse.tile as tile
from concourse import bacc
from concourse.bass_utils import run_bass_kernel_spmd

F32 = mybir.dt.float32
BF16 = mybir.dt.bfloat16
AF = mybir.ActivationFunctionType

# Problem constants (fixed by the reference)
T, DM, NS, FEAT = 100, 100, 32, 400
H1, H2, H3, B = 256, 128, 512, 2

NCORES = 8
DB = 25               # duration rows per core block
HALO = 2              # halo rows each side (two 3x3 convs)
ROWS = DB + 2 * HALO  # 29 rows of fmap computed per core
COLS = ROWS * T       # 2900 flattened (d, s) columns per core

# N-tiles over the 2900 columns (PSUM bank = 512 f32)
NTILES = [(c0, min(500, COLS - c0)) for c0 in range(0, COLS, 500)]
NT = len(NTILES)

# bias column layout in the packed [128, 20] bias tensor
BC_B1, BC_B2, BC_S1, BC_E1, BC_P = (0, 1), (2, 3), (4, 5), (6, 7), (8, 9)
BC_3D = (10, 11, 12, 13)
BC_C1, BC_C2, BC_C3, BC_C4, BC_S2, BC_E2 = 14, 15, 16, 17, 18, 19

_CACHED_NC = None


def _bias_ap(biases_sb, col, nparts=128):
    return biases_sb[0:nparts, col:col + 1]


def _emit_conv1d(nc, pp, biases_sb, in_sb, kt_cnt, w_sb, bias_cols, out_sb, func,
                 tag):
    """conv1d(k=3, pad=1) as 3 shifted matmuls per (oc-chunk, ic-chunk).

    in_sb:  [128, kt_cnt, T] bf16, w_sb: [128, 3, kt_cnt, 256] bf16,
    out_sb: [128, 2, T].
    """
    for oc in range(2):
        ps = pp.tile([128, 512], F32, tag="ps", name=f"ps_{tag}{oc}")[:, :T]
        seq = [(dt, kt) for dt in (1, 0, 2) for kt in range(kt_cnt)]
        for idx, (dt, kt) in enumerate(seq):
            if dt == 1:
                o0, o1, i0, i1 = 0, T, 0, T
            elif dt == 0:
                o0, o1, i0, i1 = 1, T, 0, T - 1
            else:
                o0, o1, i0, i1 = 0, T - 1, 1, T
            nc.tensor.matmul(
                ps[:, o0:o1],
                lhsT=w_sb[:, dt, kt, oc * 128:(oc + 1) * 128],
                rhs=in_sb[:, kt, i0:i1],
                start=(idx == 0),
                stop=(idx == len(seq) - 1),
            )
        nc.scalar.activation(out_sb[:, oc, :], ps, func,
                             bias=_bias_ap(biases_sb, bias_cols[oc]))


def _build_module():
    nc = bacc.Bacc("TRN2", target_bir_lowering=False, debug=False)

    # ---- DRAM I/O ----
    d_xb = nc.dram_tensor("xb", [128, 4, T], BF16, kind="ExternalInput")
    d_wb1 = nc.dram_tensor("wb1", [128, 3, 4, H1], BF16, kind="ExternalInput")
    d_wb2 = nc.dram_tensor("wb2", [128, 3, 2, H1], BF16, kind="ExternalInput")
    d_ws1 = nc.dram_tensor("ws1", [128, 3, 2, H1], BF16, kind="ExternalInput")
    d_we1 = nc.dram_tensor("we1", [128, 3, 2, H1], BF16, kind="ExternalInput")
    d_wp = nc.dram_tensor("wp", [128, 3, 2, H1], BF16, kind="ExternalInput")
    d_ws2 = nc.dram_tensor("ws2", [128, 2], BF16, kind="ExternalInput")
    d_we2 = nc.dram_tensor("we2", [128, 2], BF16, kind="ExternalInput")
    d_w3dT = nc.dram_tensor("w3dT", [2, 128, NS, H3], BF16, kind="ExternalInput")
    d_mask = nc.dram_tensor("maskd", [T, NT, NS, 500], BF16,
                            kind="ExternalInput")
    d_wc1 = nc.dram_tensor("wc1", [128, 4, H2], BF16, kind="ExternalInput")
    d_wc2 = nc.dram_tensor("wc2", [128, 9, H2], BF16, kind="ExternalInput")
    d_wc3 = nc.dram_tensor("wc3", [128, 9, H2], BF16, kind="ExternalInput")
    d_wc4 = nc.dram_tensor("wc4", [128, 2], BF16, kind="ExternalInput")
    d_bias = nc.dram_tensor("biases", [128, 20], F32, kind="ExternalInput")

    d_conf = nc.dram_tensor("conf_o", [2, DB * T], F32, kind="ExternalOutput")
    d_start = nc.dram_tensor("start_o", [1, T], F32, kind="ExternalOutput")
    d_end = nc.dram_tensor("end_o", [1, T], F32, kind="ExternalOutput")
    d_fmap = nc.dram_tensor("fmap_o", [H3, DB * T], F32, kind="ExternalOutput")

    with tile.TileContext(nc) as tc:
        with (
            tc.tile_pool(name="const", bufs=1) as cp,
            tc.tile_pool(name="wpool", bufs=3) as wpo,
            tc.tile_pool(name="mpool", bufs=3) as mp,
            tc.tile_pool(name="spool", bufs=4) as sp,
            tc.tile_pool(name="psum", bufs=8, space="PSUM") as pp,
        ):
            # ---- load constants ----
            def cload(name, shape, dtype, dram):
                t = cp.tile(shape, dtype, name=name)
                nc.sync.dma_start(t[:], dram[:])
                return t

            biases_sb = cload("biases_sb", [128, 20], F32, d_bias)
            xb_sb = cload("xb_sb", [128, 4, T], BF16, d_xb)
            wb1_sb = cload("wb1_sb", [128, 3, 4, H1], BF16, d_wb1)
            wb2_sb = cload("wb2_sb", [128, 3, 2, H1], BF16, d_wb2)
            ws1_sb = cload("ws1_sb", [128, 3, 2, H1], BF16, d_ws1)
            we1_sb = cload("we1_sb", [128, 3, 2, H1], BF16, d_we1)
            wp_sb = cload("wp_sb", [128, 3, 2, H1], BF16, d_wp)
            ws2_sb = cload("ws2_sb", [128, 2], BF16, d_ws2)
            we2_sb = cload("we2_sb", [128, 2], BF16, d_we2)
            wc1_sb = cload("wc1_sb", [128, 4, H2], BF16, d_wc1)
            wc2_sb = cload("wc2_sb", [128, 9, H2], BF16, d_wc2)
            wc3_sb = cload("wc3_sb", [128, 9, H2], BF16, d_wc3)
            wc4_sb = cload("wc4_sb", [128, 2], BF16, d_wc4)

            # Prefetch the first mask half-groups BEFORE the w3dT stream so
            # the first big-matmul N-tiles aren't queued behind 8.4MB of w3dT.
            def mask_load(ti, h):
                mg = mp.tile([128, 16, 500], BF16, tag="mg",
                             name=f"mg{ti}_{h}")
                nc.gpsimd.dma_start(mg[0:T],
                                    d_mask[:, ti, h * 16:(h + 1) * 16, :])
                return mg

            # Tiny gate read: the first gpsimd instruction depends on wb1_sb,
            # so the SWDGE mask stream can't grab the DMA engines before the
            # small trunk-critical weight loads have landed.
            gate_sb = cp.tile([1, 4], BF16, name="gate_sb")
            nc.gpsimd.tensor_copy(out=gate_sb[0:1, :],
                                  in_=wb1_sb[0:1, 0, 0, 0:4])
            mg_pre = {(0, 0): mask_load(0, 0), (0, 1): mask_load(0, 1),
                      (1, 0): mask_load(1, 0)}
            w3t_pre = {}
            _w3t0 = []
            for kt in range(2):
                wt = wpo.tile([128, 8, H3], BF16, tag="w3t",
                              name=f"w3t_0_{kt}")
                nc.sync.dma_start(wt[:], d_w3dT[kt, :, 0:8, :])
                _w3t0.append(wt)
            w3t_pre[0] = _w3t0

            # ---- persistent activations ----
            h1_sb = cp.tile([128, 2, T], BF16, name="h1_sb")
            h2_sb = cp.tile([128, 2, T], BF16, name="h2_sb")
            hs_sb = cp.tile([128, 2, T], BF16, name="hs_sb")
            he_sb = cp.tile([128, 2, T], BF16, name="he_sb")
            p_sb = cp.tile([128, 2, T], F32, name="p_sb")
            p_bf = cp.tile([128, 2, T], BF16, name="p_bf")
            head_s_sb = cp.tile([1, T], F32, name="head_s_sb")
            head_e_sb = cp.tile([1, T], F32, name="head_e_sb")
            r_ch = [cp.tile([128, 8 * H3], BF16, name=f"r_ch{c}")
                    for c in range(4)]
            fmap_bf = [cp.tile([128, COLS], BF16, name=f"fmap_bf{c}")
                       for c in range(4)]
            c1_bf = cp.tile([128, COLS], BF16, name="c1_bf")
            c2_bf = cp.tile([128, (ROWS - 2) * T], BF16, name="c2_bf")
            c3_bf = cp.tile([128, DB * T], BF16, name="c3_bf")
            conf_sb = cp.tile([2, DB * T], F32, name="conf_sb")

            # ---- 1D trunk ----
            _emit_conv1d(nc, pp, biases_sb, xb_sb, 4, wb1_sb, BC_B1, h1_sb,
                         AF.Relu, "b1")
            _emit_conv1d(nc, pp, biases_sb, h1_sb, 2, wb2_sb, BC_B2, h2_sb,
                         AF.Relu, "b2")
            _emit_conv1d(nc, pp, biases_sb, h2_sb, 2, wp_sb, BC_P, p_sb,
                         AF.Relu, "pc")
            nc.vector.tensor_copy(out=p_bf[:], in_=p_sb[:])

            # start / end heads (independent of the BM path; emitted here so
            # they fill PE gaps while w3dT streams in)
            _emit_conv1d(nc, pp, biases_sb, h2_sb, 2, ws1_sb, BC_S1, hs_sb,
                         AF.Relu, "s1")
            _emit_conv1d(nc, pp, biases_sb, h2_sb, 2, we1_sb, BC_E1, he_sb,
                         AF.Relu, "e1")
            for w2_sb, bcol, out_dram, hin, hd, tag in (
                (ws2_sb, BC_S2, d_start, hs_sb, head_s_sb, "hds"),
                (we2_sb, BC_E2, d_end, he_sb, head_e_sb, "hde"),
            ):
                ps = pp.tile([128, 512], F32, tag="ps", name=f"ps_{tag}")[:1, :T]
                for kt in range(2):
                    nc.tensor.matmul(ps, lhsT=w2_sb[:, kt:kt + 1],
                                     rhs=hin[:, kt, :],
                                     start=(kt == 0), stop=(kt == 1))
                nc.scalar.activation(hd[0:1, :], ps, AF.Sigmoid,
                                     bias=biases_sb[0:1, bcol:bcol + 1])
                nc.sync.dma_start(out_dram[:], hd[0:1, :])

            # ---- r[n](t, o) = sum_c p[c, t] * w3d[o, c, n] ----
            # w3dT streamed in chunks of 8 n-slices (8KB/partition lines);
            # chunk 0 was prefetched before the trunk.
            def emit_r_chunk(ch):
                wts = w3t_pre.pop(ch, None)
                if wts is None:
                    wts = []
                    for kt in range(2):
                        wt = wpo.tile([128, 8, H3], BF16, tag="w3t",
                                      name=f"w3t_{ch}_{kt}")
                        nc.sync.dma_start(wt[:],
                                          d_w3dT[kt, :, ch * 8:(ch + 1) * 8, :])
                        wts.append(wt)
                for j in range(8):
                    n = ch * 8 + j
                    ps = pp.tile([128, 512], F32, tag="ps",
                                 name=f"ps_r{n}")[:T, :H3]
                    for kt in range(2):
                        nc.tensor.matmul(ps, lhsT=p_bf[:, kt, :],
                                         rhs=wts[kt][:, j, :],
                                         start=(kt == 0), stop=(kt == 1))
                    nc.vector.tensor_copy(
                        out=r_ch[ch][0:T, j * H3:(j + 1) * H3], in_=ps)

            # ---- big matmul: fmap[o, (d,s)] = relu(sum_n r[n].T @ mask[n]) ----
            # Half-group-major accumulation: 4 PSUM banks accumulate all 4
            # oc-chunks over n=0..15 (half 0), then n=16..31 (half 1), so each
            # mask half-group's SBUF slot frees mid-tile for deeper prefetch.
            # Half 0 only needs r chunks 0-1, so tile 0's half 0 is emitted
            # between r chunks to keep PE dense while w3dT streams.
            def bm_alloc(ti):
                w = NTILES[ti][1]
                return [pp.tile([128, 512], F32, tag="ps",
                                name=f"ps_bm{ti}_{oc}")[:, :w] for oc in range(4)]

            def bm_half(ti, h, pss):
                w = NTILES[ti][1]
                mg = mg_pre.pop((ti, h), None)
                if mg is None:
                    mg = mask_load(ti, h)
                for oc in range(4):
                    for j in range(16):
                        n = h * 16 + j
                        nc.tensor.matmul(
                            pss[oc],
                            lhsT=r_ch[n // 8][0:T, (n % 8) * H3 + oc * 128:
                                              (n % 8) * H3 + (oc + 1) * 128],
                            rhs=mg[0:T, j, :w],
                            start=(n == 0), stop=(n == NS - 1))

            def bm_evict(ti, pss):
                c0, w = NTILES[ti]
                for oc in range(4):
                    st = sp.tile([128, 500], F32, tag="st",
                                 name=f"st{ti}_{oc}")[:, :w]
                    nc.scalar.activation(st, pss[oc], AF.Relu,
                                         bias=_bias_ap(biases_sb, BC_3D[oc]))
                    nc.vector.tensor_copy(out=fmap_bf[oc][:, c0:c0 + w], in_=st)
                    lo, hi = max(c0, HALO * T), min(c0 + w, (HALO + DB) * T)
                    if lo < hi:
                        nc.sync.dma_start(
                            d_fmap[oc * 128:(oc + 1) * 128,
                                   lo - HALO * T:hi - HALO * T],
                            st[:, lo - c0:hi - c0])

            emit_r_chunk(0)
            emit_r_chunk(1)
            pss0 = bm_alloc(0)
            bm_half(0, 0, pss0)
            emit_r_chunk(2)
            emit_r_chunk(3)
            bm_half(0, 1, pss0)
            bm_evict(0, pss0)
            for ti in range(1, NT):
                pss = bm_alloc(ti)
                bm_half(ti, 0, pss)
                bm_half(ti, 1, pss)
                bm_evict(ti, pss)

            # ---- c1: 1x1 conv H3 -> H2, relu ----
            for ti, (c0, w) in enumerate(NTILES):
                ps = pp.tile([128, 512], F32, tag="ps", name=f"ps_c1{ti}")[:, :w]
                for kt in range(4):
                    nc.tensor.matmul(ps, lhsT=wc1_sb[:, kt, :],
                                     rhs=fmap_bf[kt][:, c0:c0 + w],
                                     start=(kt == 0), stop=(kt == 3))
                nc.scalar.activation(c1_bf[:, c0:c0 + w], ps, AF.Relu,
                                     bias=_bias_ap(biases_sb, BC_C1))

            # ---- c2 / c3: 3x3 convs (pad=1), relu ----
            # Blocks of 5 output rows per PSUM bank: dy-shifts are whole-row
            # shifts (one wide matmul); dx-shifts need per-row edge-limited
            # matmuls to respect the s-boundary zero padding.
            for w_sb, in_bf, out_bf, nrows, bcol, tag in (
                (wc2_sb, c1_bf, c2_bf, ROWS - 2, BC_C2, "c2"),
                (wc3_sb, c2_bf, c3_bf, ROWS - 4, BC_C3, "c3"),
            ):
                for r0 in range(0, nrows, 5):
                    r1 = min(r0 + 5, nrows)
                    nb = r1 - r0
                    ps = pp.tile([128, 512], F32, tag="ps",
                                 name=f"ps_{tag}_{r0}")[:, :nb * T]
                    first = True
                    for dy in (-1, 0, 1):
                        for dx in (0, -1, 1):
                            widx = (dy + 1) * 3 + (dx + 1)
                            last = (dy == 1 and dx == 1)
                            if dx == 0:
                                i0 = (r0 + 1 + dy) * T
                                nc.tensor.matmul(
                                    ps, lhsT=w_sb[:, widx, :],
                                    rhs=in_bf[:, i0:i0 + nb * T],
                                    start=first, stop=False)
                                first = False
                            else:
                                o0, o1 = (1, T) if dx == -1 else (0, T - 1)
                                i0, i1 = (0, T - 1) if dx == -1 else (1, T)
                                for r in range(r0, r1):
                                    ib = (r + 1 + dy) * T
                                    ob = (r - r0) * T
                                    nc.tensor.matmul(
                                        ps[:, ob + o0:ob + o1],
                                        lhsT=w_sb[:, widx, :],
                                        rhs=in_bf[:, ib + i0:ib + i1],
                                        start=False,
                                        stop=last and r == r1 - 1)
                    nc.scalar.activation(out_bf[:, r0 * T:r1 * T], ps,
                                         AF.Relu, bias=_bias_ap(biases_sb, bcol))

            # ---- c4: 1x1 conv H2 -> 2, sigmoid -> conf ----
            for ti in range(5):
                c0 = ti * 500
                ps = pp.tile([128, 512], F32, tag="ps", name=f"ps_c4{ti}")[:2, :500]
                nc.tensor.matmul(ps, lhsT=wc4_sb[:], rhs=c3_bf[:, c0:c0 + 500],
                                 start=True, stop=True)
                nc.scalar.activation(conf_sb[:, c0:c0 + 500], ps, AF.Sigmoid,
                                     bias=_bias_ap(biases_sb, BC_C4, nparts=2))
            nc.sync.dma_start(d_conf[:], conf_sb[:])

    nc.compile()
    return nc


def get_module():
    global _CACHED_NC
    if _CACHED_NC is None:
        _CACHED_NC = _build_module()
    return _CACHED_NC


# ---------------- host-side prep ----------------

def _bf16(a):
    return np.asarray(a, dtype=np.float32).astype(ml_dtypes.bfloat16)


def _dense_grouped(w, groups):
    """[oc, ic_g, k] grouped conv weight -> dense [oc, ic, k] block-diagonal."""
    oc, icg, k = w.shape
    ocg = oc // groups
    dense = np.zeros((oc, icg * groups, k), np.float32)
    for g in range(groups):
        dense[g * ocg:(g + 1) * ocg, g * icg:(g + 1) * icg, :] = \
            w[g * ocg:(g + 1) * ocg]
    return dense


def _pack_conv1d(w_dense, kt_cnt):
    """[oc, ic, 3] -> [128, 3, kt_cnt, oc] with ic = kt*128 + p (zero-padded)."""
    oc, ic, k = w_dense.shape
    arr = np.zeros((128, 3, kt_cnt, oc), np.float32)
    wt = w_dense.transpose(2, 1, 0)  # [3, ic, oc]
    pad = np.zeros((3, kt_cnt * 128, oc), np.float32)
    pad[:, :ic] = wt
    arr[:] = pad.reshape(3, kt_cnt, 128, oc).transpose(2, 0, 1, 3)
    return arr


def _prep_in_maps(inputs):
    inp = {k: np.asarray(v, np.float32) for k, v in inputs.items()}
    x = inp["x"]

    wb1 = _bf16(_pack_conv1d(_dense_grouped(inp["w_b1"], 4), 4))
    wb2 = _bf16(_pack_conv1d(_dense_grouped(inp["w_b2"], 4), 2))
    ws1 = _bf16(_pack_conv1d(_dense_grouped(inp["w_s1"], 4), 2))
    we1 = _bf16(_pack_conv1d(_dense_grouped(inp["w_e1"], 4), 2))
    wpk = _bf16(_pack_conv1d(inp["w_p"], 2))
    ws2 = _bf16(inp["w_s2"].reshape(H1).reshape(2, 128).T)
    we2 = _bf16(inp["w_e2"].reshape(H1).reshape(2, 128).T)
    w3dT = _bf16(inp["w_3d"].transpose(1, 2, 0).reshape(2, 128, NS, H3))
    wc1 = _bf16(inp["w_c1"].reshape(H2, H3).T.reshape(4, 128, H2)
                .transpose(1, 0, 2))
    wc2 = _bf16(inp["w_c2"].transpose(2, 3, 1, 0).reshape(9, H2, H2)
                .transpose(1, 0, 2))
    wc3 = _bf16(inp["w_c3"].transpose(2, 3, 1, 0).reshape(9, H2, H2)
                .transpose(1, 0, 2))
    wc4 = _bf16(inp["w_c4"].reshape(2, H2).T)

    biases = np.zeros((128, 20), np.float32)
    for (c0, c1), b in ((BC_B1, inp["b_b1"]), (BC_B2, inp["b_b2"]),
                        (BC_S1, inp["b_s1"]), (BC_E1, inp["b_e1"]),
                        (BC_P, inp["b_p"])):
        biases[:, c0] = b[:128]
        biases[:, c1] = b[128:]
    for i, c in enumerate(BC_3D):
        biases[:, c] = inp["b_3d"][i * 128:(i + 1) * 128]
    biases[:, BC_C1] = inp["b_c1"]
    biases[:, BC_C2] = inp["b_c2"]
    biases[:, BC_C3] = inp["b_c3"]
    biases[0:2, BC_C4] = inp["b_c4"]
    biases[0, BC_S2] = inp["b_s2"][0]
    biases[0, BC_E2] = inp["b_e2"][0]

    # per-batch x, padded to 512 input channels, ic-chunked
    xbs = []
    for b in range(B):
        xp = np.zeros((512, T), np.float32)
        xp[:FEAT] = x[b]
        xbs.append(_bf16(xp.reshape(4, 128, T).transpose(1, 0, 2)))

    # per-block mask slices (with halo, zero-padded at the d edges),
    # packed [T, ntile, NS, 500] bf16 for wide DMA lines
    m5 = np.asarray(inp["sample_mask"], np.float32).reshape(T, NS, DM, T)
    masks = []
    for j in range(4):
        sl = np.zeros((T, NS, ROWS, T), np.float32)
        d0 = DB * j - HALO
        lo, hi = max(d0, 0), min(d0 + ROWS, DM)
        sl[:, :, lo - d0:hi - d0, :] = m5[:, :, lo:hi, :]
        flat = sl.reshape(T, NS, COLS)
        packed = np.zeros((T, NT, NS, 500), np.float32)
        for ti, (c0, w) in enumerate(NTILES):
            packed[:, ti, :, :w] = flat[:, :, c0:c0 + w]
        masks.append(_bf16(packed))

    shared = dict(wb1=wb1, wb2=wb2, ws1=ws1, we1=we1, wp=wpk, ws2=ws2, we2=we2,
                  w3dT=w3dT, wc1=wc1, wc2=wc2, wc3=wc3, wc4=wc4, biases=biases)
    in_maps = []
    for i in range(NCORES):
        b, j = i // 4, i % 4
        in_maps.append(dict(shared, xb=xbs[b], maskd=masks[j]))
    return in_maps


def _gather(results):
    conf = np.zeros((B, 2, DM, T), np.float32)
    fmap = np.zeros((B, H3, DM, T), np.float32)
    start = np.zeros((B, T), np.float32)
    end = np.zeros((B, T), np.float32)
    for i, res in enumerate(results):
        b, j = i // 4, i % 4
        conf[b, :, DB * j:DB * (j + 1), :] = res["conf_o"].reshape(2, DB, T)
        fmap[b, :, DB * j:DB * (j + 1), :] = res["fmap_o"].reshape(H3, DB, T)
        if j == 0:
            start[b] = res["start_o"][0]
            end[b] = res["end_o"][0]
    return conf, start, end, fmap


def run(inputs, trace=False):
    nc = get_module()
    in_maps = _prep_in_maps(inputs)
    res = run_bass_kernel_spmd(nc, in_maps, core_ids=list(range(NCORES)),
                               trace=trace)
    return _gather(res.results), res


def kernel(**inputs):
    outs, _ = run(inputs, trace=False)
    return outs


# revision 20
# speedup vs baseline: 1.3014x; 1.3014x over previous
"""BMN (Boundary Matching Network) forward pass as a Bass/Tile kernel on 8 trn2 cores.

The BM map lives on a [DM=100, T=100] (duration d, start s) grid whose useful
region is the triangle d+s < 100 (the reference's sample_mask zeroes the rest,
so fmap/conf are constants there). We shard that triangle into 4 congruent
pieces per batch (8 cores = 2 batches x 4 pieces):

    A: d in [0,50),  s in [0, 50-d)      (corner triangle)
    B: d in [50,100), s in [0, 100-d)    (= A translated by +50 in d)
    C: d in [0,50),  s in [50, 100-d)    (= A translated by +50 in s)
    D: d in [0,50),  s in [50-d, 50)     (= A rotated 180 deg)

Every piece is stored as the same "staircase" layout: 54 rows, row r holding
W(r) = clamp(60-r, 8, 58) columns (piece + 2-cell conv halo + 2-cell fringe +
2 spare). Piece D runs with 180-degree-rotated conv2d weights and mask columns
(host-side data remap only; the instruction stream is identical on all cores).
Cells of the grid more than 2 away from the valid triangle are filled on the
host with the closed-form constant the reference produces there.

Key reformulation of the BM layer: instead of bm = p @ mask then a (c,n)
contraction, we contract w_3d with p first (r[n] = p.T @ w3d[:,:,n], 32 small
matmuls) and compute fmap[o, cell] = relu(sum_n r[n].T @ mask[n]) with the
contraction accumulated in PSUM. This avoids materializing bm and shrinks the
contraction from 8192 to 3200.

All matmuls run in bf16 with fp32 PSUM accumulation; mask/weights are host
pre-cast to bf16 (round-to-nearest-even, same as an on-device cast). The mask
is host-packed as [T, ntile, NS, 500] so each N-tile half-group loads with
16KB-contiguous per-partition DMA lines. Grouped conv1ds are emitted as dense
matmuls with block-diagonal weights.
"""

import numpy as np
import ml_dtypes

import concourse.bass as bass
import concourse.mybir as mybir
import concourse.tile as tile
from concourse import bacc
from concourse.bass_utils import run_bass_kernel_spmd

F32 = mybir.dt.float32
BF16 = mybir.dt.bfloat16
AF = mybir.ActivationFunctionType

# Problem constants (fixed by the reference)
T, DM, NS, FEAT = 100, 100, 32, 400
H1, H2, H3, B = 256, 128, 512, 2

NCORES = 8
R = 54                                     # stored staircase rows per piece
WIDTHS = [min(58, max(8, 60 - r)) for r in range(R)]
OFF = np.concatenate([[0], np.cumsum(WIDTHS)]).astype(int)
CP = int(OFF[-1])                          # packed columns per core

# N-tiles over the packed columns (PSUM bank = 512 f32)
NTILES = [(c0, min(500, CP - c0)) for c0 in range(0, CP, 500)]
NT = len(NTILES)

# bias column layout in the packed [128, 20] bias tensor
BC_B1, BC_B2, BC_S1, BC_E1, BC_P = (0, 1), (2, 3), (4, 5), (6, 7), (8, 9)
BC_3D = (10, 11, 12, 13)
BC_C1, BC_C2, BC_C3, BC_C4, BC_S2, BC_E2 = 14, 15, 16, 17, 18, 19

_CACHED_NC = None


def _bias_ap(biases_sb, col, nparts=128):
    return biases_sb[0:nparts, col:col + 1]


def _emit_conv1d(nc, pp, biases_sb, in_sb, kt_cnt, w_sb, bias_cols, out_sb, func,
                 tag):
    """conv1d(k=3, pad=1) as 3 shifted matmuls per (oc-chunk, ic-chunk)."""
    for oc in range(2):
        ps = pp.tile([128, 512], F32, tag="ps", name=f"ps_{tag}{oc}")[:, :T]
        seq = [(dt, kt) for dt in (1, 0, 2) for kt in range(kt_cnt)]
        for idx, (dt, kt) in enumerate(seq):
            if dt == 1:
                o0, o1, i0, i1 = 0, T, 0, T
            elif dt == 0:
                o0, o1, i0, i1 = 1, T, 0, T - 1
            else:
                o0, o1, i0, i1 = 0, T - 1, 1, T
            nc.tensor.matmul(
                ps[:, o0:o1],
                lhsT=w_sb[:, dt, kt, oc * 128:(oc + 1) * 128],
                rhs=in_sb[:, kt, i0:i1],
                start=(idx == 0),
                stop=(idx == len(seq) - 1),
            )
        nc.scalar.activation(out_sb[:, oc, :], ps, func,
                             bias=_bias_ap(biases_sb, bias_cols[oc]))


def _build_module():
    nc = bacc.Bacc("TRN2", target_bir_lowering=False, debug=False)

    # ---- DRAM I/O ----
    d_xb = nc.dram_tensor("xb", [128, 4, T], BF16, kind="ExternalInput")
    d_wb1 = nc.dram_tensor("wb1", [128, 3, 4, H1], BF16, kind="ExternalInput")
    d_wb2 = nc.dram_tensor("wb2", [128, 3, 2, H1], BF16, kind="ExternalInput")
    d_ws1 = nc.dram_tensor("ws1", [128, 3, 2, H1], BF16, kind="ExternalInput")
    d_we1 = nc.dram_tensor("we1", [128, 3, 2, H1], BF16, kind="ExternalInput")
    d_wp = nc.dram_tensor("wp", [128, 3, 2, H1], BF16, kind="ExternalInput")
    d_ws2 = nc.dram_tensor("ws2", [128, 2], BF16, kind="ExternalInput")
    d_we2 = nc.dram_tensor("we2", [128, 2], BF16, kind="ExternalInput")
    d_w3dT = nc.dram_tensor("w3dT", [2, 128, NS, H3], BF16, kind="ExternalInput")
    d_mask = nc.dram_tensor("maskd", [T, NT, NS, 500], BF16,
                            kind="ExternalInput")
    d_wc1 = nc.dram_tensor("wc1", [128, 4, H2], BF16, kind="ExternalInput")
    d_wc2 = nc.dram_tensor("wc2", [128, 9, H2], BF16, kind="ExternalInput")
    d_wc3 = nc.dram_tensor("wc3", [128, 9, H2], BF16, kind="ExternalInput")
    d_wc4 = nc.dram_tensor("wc4", [128, 2], BF16, kind="ExternalInput")
    d_bias = nc.dram_tensor("biases", [128, 20], F32, kind="ExternalInput")

    d_conf = nc.dram_tensor("conf_o", [2, CP], F32, kind="ExternalOutput")
    d_start = nc.dram_tensor("start_o", [1, T], F32, kind="ExternalOutput")
    d_end = nc.dram_tensor("end_o", [1, T], F32, kind="ExternalOutput")
    d_fmap = nc.dram_tensor("fmap_o", [H3, CP], F32, kind="ExternalOutput")

    with tile.TileContext(nc) as tc:
        with (
            tc.tile_pool(name="const", bufs=1) as cp,
            tc.tile_pool(name="wpool", bufs=3) as wpo,
            tc.tile_pool(name="mpool", bufs=3) as mp,
            tc.tile_pool(name="spool", bufs=4) as sp,
            tc.tile_pool(name="psum", bufs=8, space="PSUM") as pp,
        ):
            # ---- load constants ----
            def cload(name, shape, dtype, dram):
                t = cp.tile(shape, dtype, name=name)
                nc.sync.dma_start(t[:], dram[:])
                return t

            biases_sb = cload("biases_sb", [128, 20], F32, d_bias)
            xb_sb = cload("xb_sb", [128, 4, T], BF16, d_xb)
            wb1_sb = cload("wb1_sb", [128, 3, 4, H1], BF16, d_wb1)
            wb2_sb = cload("wb2_sb", [128, 3, 2, H1], BF16, d_wb2)
            ws1_sb = cload("ws1_sb", [128, 3, 2, H1], BF16, d_ws1)
            we1_sb = cload("we1_sb", [128, 3, 2, H1], BF16, d_we1)
            wp_sb = cload("wp_sb", [128, 3, 2, H1], BF16, d_wp)
            ws2_sb = cload("ws2_sb", [128, 2], BF16, d_ws2)
            we2_sb = cload("we2_sb", [128, 2], BF16, d_we2)
            wc1_sb = cload("wc1_sb", [128, 4, H2], BF16, d_wc1)
            wc2_sb = cload("wc2_sb", [128, 9, H2], BF16, d_wc2)
            wc3_sb = cload("wc3_sb", [128, 9, H2], BF16, d_wc3)
            wc4_sb = cload("wc4_sb", [128, 2], BF16, d_wc4)

            # Prefetch the first mask half-groups on the gpsimd (SWDGE) queue
            # so they stream in parallel with the w3dT loads on the sync queue.
            def mask_load(ti, h):
                mg = mp.tile([128, 16, 500], BF16, tag="mg",
                             name=f"mg{ti}_{h}")
                nc.gpsimd.dma_start(mg[0:T],
                                    d_mask[:, ti, h * 16:(h + 1) * 16, :])
                return mg

            mg_pre = {(0, 0): mask_load(0, 0), (0, 1): mask_load(0, 1),
                      (1, 0): mask_load(1, 0)}
            w3t_pre = {}
            _w3t0 = []
            for kt in range(2):
                wt = wpo.tile([128, 8, H3], BF16, tag="w3t",
                              name=f"w3t_0_{kt}")
                nc.sync.dma_start(wt[:], d_w3dT[kt, :, 0:8, :])
                _w3t0.append(wt)
            w3t_pre[0] = _w3t0

            # ---- persistent activations ----
            h1_sb = cp.tile([128, 2, T], BF16, name="h1_sb")
            h2_sb = cp.tile([128, 2, T], BF16, name="h2_sb")
            hs_sb = cp.tile([128, 2, T], BF16, name="hs_sb")
            he_sb = cp.tile([128, 2, T], BF16, name="he_sb")
            p_sb = cp.tile([128, 2, T], F32, name="p_sb")
            p_bf = cp.tile([128, 2, T], BF16, name="p_bf")
            head_s_sb = cp.tile([1, T], F32, name="head_s_sb")
            head_e_sb = cp.tile([1, T], F32, name="head_e_sb")
            r_ch = [cp.tile([128, 8 * H3], BF16, name=f"r_ch{c}")
                    for c in range(4)]
            fmap_bf = [cp.tile([128, CP], BF16, name=f"fmap_bf{c}")
                       for c in range(4)]
            c1_bf = cp.tile([128, CP], BF16, name="c1_bf")
            c2_bf = cp.tile([128, CP], BF16, name="c2_bf")
            c3_bf = cp.tile([128, CP], BF16, name="c3_bf")
            conf_sb = cp.tile([2, CP], F32, name="conf_sb")

            # rows outside the written ranges of c2/c3 are never computed;
            # zero them so downstream reads are defined
            nc.vector.memset(c2_bf[:], 0.0)
            nc.vector.memset(c3_bf[:], 0.0)

            # ---- 1D trunk ----
            _emit_conv1d(nc, pp, biases_sb, xb_sb, 4, wb1_sb, BC_B1, h1_sb,
                         AF.Relu, "b1")
            _emit_conv1d(nc, pp, biases_sb, h1_sb, 2, wb2_sb, BC_B2, h2_sb,
                         AF.Relu, "b2")
            _emit_conv1d(nc, pp, biases_sb, h2_sb, 2, wp_sb, BC_P, p_sb,
                         AF.Relu, "pc")
            nc.vector.tensor_copy(out=p_bf[:], in_=p_sb[:])

            # start / end heads (independent of the BM path)
            _emit_conv1d(nc, pp, biases_sb, h2_sb, 2, ws1_sb, BC_S1, hs_sb,
                         AF.Relu, "s1")
            _emit_conv1d(nc, pp, biases_sb, h2_sb, 2, we1_sb, BC_E1, he_sb,
                         AF.Relu, "e1")
            for w2_sb, bcol, out_dram, hin, hd, tag in (
                (ws2_sb, BC_S2, d_start, hs_sb, head_s_sb, "hds"),
                (we2_sb, BC_E2, d_end, he_sb, head_e_sb, "hde"),
            ):
                ps = pp.tile([128, 512], F32, tag="ps", name=f"ps_{tag}")[:1, :T]
                for kt in range(2):
                    nc.tensor.matmul(ps, lhsT=w2_sb[:, kt:kt + 1],
                                     rhs=hin[:, kt, :],
                                     start=(kt == 0), stop=(kt == 1))
                nc.scalar.activation(hd[0:1, :], ps, AF.Sigmoid,
                                     bias=biases_sb[0:1, bcol:bcol + 1])
                nc.sync.dma_start(out_dram[:], hd[0:1, :])

            # ---- r[n](t, o) = sum_c p[c, t] * w3d[o, c, n] ----
            def emit_r_chunk(ch):
                wts = w3t_pre.pop(ch, None)
                if wts is None:
                    wts = []
                    for kt in range(2):
                        wt = wpo.tile([128, 8, H3], BF16, tag="w3t",
                                      name=f"w3t_{ch}_{kt}")
                        nc.sync.dma_start(wt[:],
                                          d_w3dT[kt, :, ch * 8:(ch + 1) * 8, :])
                        wts.append(wt)
                for j in range(8):
                    n = ch * 8 + j
                    ps = pp.tile([128, 512], F32, tag="ps",
                                 name=f"ps_r{n}")[:T, :H3]
                    for kt in range(2):
                        nc.tensor.matmul(ps, lhsT=p_bf[:, kt, :],
                                         rhs=wts[kt][:, j, :],
                                         start=(kt == 0), stop=(kt == 1))
                    nc.vector.tensor_copy(
                        out=r_ch[ch][0:T, j * H3:(j + 1) * H3], in_=ps)

            # ---- big matmul: fmap[o, cell] = relu(sum_n r[n].T @ mask[n]) ----
            def bm_alloc(ti):
                w = NTILES[ti][1]
                return [pp.tile([128, 512], F32, tag="ps",
                                name=f"ps_bm{ti}_{oc}")[:, :w] for oc in range(4)]

            def bm_half(ti, h, pss):
                w = NTILES[ti][1]
                mg = mg_pre.pop((ti, h), None)
                if mg is None:
                    mg = mask_load(ti, h)
                for oc in range(4):
                    for j in range(16):
                        n = h * 16 + j
                        nc.tensor.matmul(
                            pss[oc],
                            lhsT=r_ch[n // 8][0:T, (n % 8) * H3 + oc * 128:
                                              (n % 8) * H3 + (oc + 1) * 128],
                            rhs=mg[0:T, j, :w],
                            start=(n == 0), stop=(n == NS - 1))

            def bm_evict(ti, pss):
                c0, w = NTILES[ti]
                for oc in range(4):
                    st = sp.tile([128, 500], F32, tag="st",
                                 name=f"st{ti}_{oc}")[:, :w]
                    nc.scalar.activation(st, pss[oc], AF.Relu,
                                         bias=_bias_ap(biases_sb, BC_3D[oc]))
                    nc.vector.tensor_copy(out=fmap_bf[oc][:, c0:c0 + w], in_=st)
                    nc.sync.dma_start(d_fmap[oc * 128:(oc + 1) * 128, c0:c0 + w],
                                      st)

            emit_r_chunk(0)
            emit_r_chunk(1)
            pss0 = bm_alloc(0)
            bm_half(0, 0, pss0)
            emit_r_chunk(2)
            emit_r_chunk(3)
            bm_half(0, 1, pss0)
            bm_evict(0, pss0)
            for ti in range(1, NT):
                pss = bm_alloc(ti)
                bm_half(ti, 0, pss)
                bm_half(ti, 1, pss)
                bm_evict(ti, pss)

            # ---- c1: 1x1 conv H3 -> H2, relu ----
            for ti, (c0, w) in enumerate(NTILES):
                ps = pp.tile([128, 512], F32, tag="ps", name=f"ps_c1{ti}")[:, :w]
                for kt in range(4):
                    nc.tensor.matmul(ps, lhsT=wc1_sb[:, kt, :],
                                     rhs=fmap_bf[kt][:, c0:c0 + w],
                                     start=(kt == 0), stop=(kt == 3))
                nc.scalar.activation(c1_bf[:, c0:c0 + w], ps, AF.Relu,
                                     bias=_bias_ap(biases_sb, BC_C1))

            # ---- c2 / c3: 3x3 convs (pad=1) over the staircase layout ----
            # Rows have varying widths/offsets, so shifts are per-row matmuls;
            # weights stay loaded across a block of 8 rows (8 PSUM banks) to
            # amortize LDWEIGHTS.
            for w_sb, in_bf, out_bf, rlo, rhi, bcol, tag in (
                (wc2_sb, c1_bf, c2_bf, 1, 53, BC_C2, "c2"),
                (wc3_sb, c2_bf, c3_bf, 2, 52, BC_C3, "c3"),
            ):
                rows = list(range(rlo, rhi))
                for b0 in range(0, len(rows), 8):
                    blk = rows[b0:b0 + 8]
                    pss = {}
                    for rr in blk:
                        pss[rr] = pp.tile([128, 512], F32, tag="ps",
                                          name=f"ps_{tag}_{rr}")[:, :WIDTHS[rr]]
                    for dy in (-1, 0, 1):
                        for dx in (0, -1, 1):
                            widx = (dy + 1) * 3 + (dx + 1)
                            for ri, rr in enumerate(blk):
                                wdt = WIDTHS[rr]
                                if dx == 0:
                                    o0, o1 = 0, wdt
                                elif dx == -1:
                                    o0, o1 = 1, wdt
                                else:
                                    o0, o1 = 0, wdt - 1
                                ib = int(OFF[rr + dy])
                                nc.tensor.matmul(
                                    pss[rr][:, o0:o1],
                                    lhsT=w_sb[:, widx, :],
                                    rhs=in_bf[:, ib + o0 + dx:ib + o1 + dx],
                                    start=(dy == -1 and dx == 0),
                                    stop=(dy == 1 and dx == 1))
                    for rr in blk:
                        nc.scalar.activation(
                            out_bf[:, int(OFF[rr]):int(OFF[rr]) + WIDTHS[rr]],
                            pss[rr], AF.Relu, bias=_bias_ap(biases_sb, bcol))

            # ---- c4: 1x1 conv H2 -> 2, sigmoid -> conf ----
            for ti, (c0, w) in enumerate(NTILES):
                ps = pp.tile([128, 512], F32, tag="ps", name=f"ps_c4{ti}")[:2, :w]
                nc.tensor.matmul(ps, lhsT=wc4_sb[:], rhs=c3_bf[:, c0:c0 + w],
                                 start=True, stop=True)
                nc.scalar.activation(conf_sb[:, c0:c0 + w], ps, AF.Sigmoid,
                                     bias=_bias_ap(biases_sb, BC_C4, nparts=2))
            nc.sync.dma_start(d_conf[:], conf_sb[:])

    nc.compile()
    return nc


def get_module():
    global _CACHED_NC
    if _CACHED_NC is None:
        _CACHED_NC = _build_module()
    return _CACHED_NC


# ---------------- piece geometry (host side) ----------------

def _piece_maps():
    """Per piece q: d_map/s_map rows of original coords for stored cells, and
    take[r][c] bools marking this core's output cells (piece + fringe,
    in-grid, mutually disjoint across pieces)."""
    maps = []
    for q in range(4):
        d_rows, s_rows, take_rows = [], [], []
        for r in range(R):
            pr = r - 2
            w = WIDTHS[r]
            sl = np.arange(w) - 2          # local s coordinate
            if q == 0:    # A
                d = np.full(w, pr)
                s = sl.copy()
                take = (sl >= 0) & (sl < 50 - pr) & (pr >= 0) & (pr < 50)
            elif q == 1:  # B
                d = np.full(w, 50 + pr)
                s = sl.copy()
                take = (sl >= 0) & (s < 102 - (50 + pr)) & (s < 100) & \
                       (pr >= 0) & (pr < 50)
            elif q == 2:  # C
                d = np.full(w, pr)
                s = 50 + sl
                take = (s >= 50) & (s < 102 - pr) & (s < 100) & \
                       (pr >= 0) & (pr < 50)
            else:         # D (180-degree rotated)
                d = np.full(w, 49 - pr)
                s = 49 - sl
                take = (pr >= 0) & (pr < 50) & (sl >= 0) & (sl < 49 - pr)
            d_rows.append(d)
            s_rows.append(s)
            take_rows.append(take)
        maps.append((d_rows, s_rows, take_rows))
    return maps


_MAPS = _piece_maps()


# ---------------- host-side prep ----------------

def _bf16(a):
    return np.asarray(a, dtype=np.float32).astype(ml_dtypes.bfloat16)


def _dense_grouped(w, groups):
    """[oc, ic_g, k] grouped conv weight -> dense [oc, ic, k] block-diagonal."""
    oc, icg, k = w.shape
    ocg = oc // groups
    dense = np.zeros((oc, icg * groups, k), np.float32)
    for g in range(groups):
        dense[g * ocg:(g + 1) * ocg, g * icg:(g + 1) * icg, :] = \
            w[g * ocg:(g + 1) * ocg]
    return dense


def _pack_conv1d(w_dense, kt_cnt):
    """[oc, ic, 3] -> [128, 3, kt_cnt, oc] with ic = kt*128 + p (zero-padded)."""
    oc, ic, k = w_dense.shape
    wt = w_dense.transpose(2, 1, 0)  # [3, ic, oc]
    pad = np.zeros((3, kt_cnt * 128, oc), np.float32)
    pad[:, :ic] = wt
    return pad.reshape(3, kt_cnt, 128, oc).transpose(2, 0, 1, 3).copy()


def _pack_conv2d(w):
    """[oc=128, ic=128, 3, 3] -> [128, 9, 128] lhsT per (ky, kx)."""
    return _bf16(w.transpose(2, 3, 1, 0).reshape(9, H2, H2).transpose(1, 0, 2))


def _deep_fill(inp):
    """Constant conf/fmap values deep inside the invalid (d+s>=100) region."""
    fmap_v = np.maximum(inp["b_3d"], 0.0)                       # [512]
    c1_v = np.maximum(inp["b_c1"] +
                      inp["w_c1"].reshape(H2, H3) @ fmap_v, 0.0)
    c2_v = np.maximum(inp["b_c2"] +
                      inp["w_c2"].sum(axis=(2, 3)) @ c1_v, 0.0)
    c3_v = np.maximum(inp["b_c3"] +
                      inp["w_c3"].sum(axis=(2, 3)) @ c2_v, 0.0)
    conf_v = 1.0 / (1.0 + np.exp(-(inp["b_c4"] +
                                   inp["w_c4"].reshape(2, H2) @ c3_v)))
    return conf_v, fmap_v


def _prep_in_maps(inputs):
    inp = {k: np.asarray(v, np.float32) for k, v in inputs.items()}
    x = inp["x"]

    wb1 = _bf16(_pack_conv1d(_dense_grouped(inp["w_b1"], 4), 4))
    wb2 = _bf16(_pack_conv1d(_dense_grouped(inp["w_b2"], 4), 2))
    ws1 = _bf16(_pack_conv1d(_dense_grouped(inp["w_s1"], 4), 2))
    we1 = _bf16(_pack_conv1d(_dense_grouped(inp["w_e1"], 4), 2))
    wpk = _bf16(_pack_conv1d(inp["w_p"], 2))
    ws2 = _bf16(inp["w_s2"].reshape(H1).reshape(2, 128).T)
    we2 = _bf16(inp["w_e2"].reshape(H1).reshape(2, 128).T)
    w3dT = _bf16(inp["w_3d"].transpose(1, 2, 0).reshape(2, 128, NS, H3))
    wc1 = _bf16(inp["w_c1"].reshape(H2, H3).T.reshape(4, 128, H2)
                .transpose(1, 0, 2))
    wc2 = _pack_conv2d(inp["w_c2"])
    wc3 = _pack_conv2d(inp["w_c3"])
    wc2_rot = _pack_conv2d(inp["w_c2"][:, :, ::-1, ::-1])
    wc3_rot = _pack_conv2d(inp["w_c3"][:, :, ::-1, ::-1])
    wc4 = _bf16(inp["w_c4"].reshape(2, H2).T)

    biases = np.zeros((128, 20), np.float32)
    for (c0, c1), b in ((BC_B1, inp["b_b1"]), (BC_B2, inp["b_b2"]),
                        (BC_S1, inp["b_s1"]), (BC_E1, inp["b_e1"]),
                        (BC_P, inp["b_p"])):
        biases[:, c0] = b[:128]
        biases[:, c1] = b[128:]
    for i, c in enumerate(BC_3D):
        biases[:, c] = inp["b_3d"][i * 128:(i + 1) * 128]
    biases[:, BC_C1] = inp["b_c1"]
    biases[:, BC_C2] = inp["b_c2"]
    biases[:, BC_C3] = inp["b_c3"]
    biases[0:2, BC_C4] = inp["b_c4"]
    biases[0, BC_S2] = inp["b_s2"][0]
    biases[0, BC_E2] = inp["b_e2"][0]

    # per-batch x, padded to 512 input channels, ic-chunked
    xbs = []
    for b in range(B):
        xp = np.zeros((512, T), np.float32)
        xp[:FEAT] = x[b]
        xbs.append(_bf16(xp.reshape(4, 128, T).transpose(1, 0, 2)))

    # per-piece mask columns, packed [T, ntile, NS, 500] bf16
    m5 = np.asarray(inp["sample_mask"], np.float32).reshape(T, NS, DM, T)
    masks = []
    for q in range(4):
        d_rows, s_rows, _ = _MAPS[q]
        flat = np.zeros((T, NS, CP), np.float32)
        for r in range(R):
            d, s = d_rows[r], s_rows[r]
            ok = (d >= 0) & (d < DM) & (s >= 0) & (s < T)
            dc, sc = np.clip(d, 0, DM - 1), np.clip(s, 0, T - 1)
            col = m5[:, :, dc, sc] * ok[None, None, :]
            flat[:, :, int(OFF[r]):int(OFF[r]) + WIDTHS[r]] = col
        packed = np.zeros((T, NT, NS, 500), np.float32)
        for ti, (c0, w) in enumerate(NTILES):
            packed[:, ti, :, :w] = flat[:, :, c0:c0 + w]
        masks.append(_bf16(packed))

    shared = dict(wb1=wb1, wb2=wb2, ws1=ws1, we1=we1, wp=wpk, ws2=ws2, we2=we2,
                  w3dT=w3dT, wc1=wc1, wc4=wc4, biases=biases)
    in_maps = []
    for i in range(NCORES):
        b, q = i // 4, i % 4
        rot = (q == 3)
        in_maps.append(dict(shared, xb=xbs[b], maskd=masks[q],
                            wc2=wc2_rot if rot else wc2,
                            wc3=wc3_rot if rot else wc3))
    return in_maps


def _gather(results, deep):
    conf_v, fmap_v = deep
    conf = np.broadcast_to(conf_v[None, :, None, None],
                           (B, 2, DM, T)).astype(np.float32).copy()
    fmap = np.broadcast_to(fmap_v[None, :, None, None],
                           (B, H3, DM, T)).astype(np.float32).copy()
    start = np.zeros((B, T), np.float32)
    end = np.zeros((B, T), np.float32)
    for i, res in enumerate(results):
        b, q = i // 4, i % 4
        d_rows, s_rows, take_rows = _MAPS[q]
        co = res["conf_o"]
        fo = res["fmap_o"]
        for r in range(R):
            tk = take_rows[r]
            if not tk.any():
                continue
            d, s = d_rows[r][tk], s_rows[r][tk]
            cols = int(OFF[r]) + np.nonzero(tk)[0]
            conf[b, :, d, s] = co[:, cols].T
            fmap[b, :, d, s] = fo[:, cols].T
        if q == 0:
            start[b] = res["start_o"][0]
            end[b] = res["end_o"][0]
    return conf, start, end, fmap


def run(inputs, trace=False):
    nc = get_module()
    in_maps = _prep_in_maps(inputs)
    deep = _deep_fill({k: np.asarray(v, np.float32) for k, v in inputs.items()})
    res = run_bass_kernel_spmd(nc, in_maps, core_ids=list(range(NCORES)),
                               trace=trace)
    return _gather(res.results, deep), res


def kernel(**inputs):
    outs, _ = run(inputs, trace=False)
    return outs
